# revision 12
# baseline (speedup 1.0000x reference)
"""Trainium2 Bass kernel for nn_NeuronCircuit_45140106281445 (MoE-routed attention).

8-core SPMD plan (v2):
  - Rank-sharded compress: core c owns rank-columns [64c, 64c+64) of the shared
    compress neuron bank and computes its Q/K/V rank slice for ALL 2048 tokens
    densely over all 32 experts (the projection is shared across the Q/K/V
    routers), then top-8 gated-combines via a broadcast-multiply plus one
    strided tensor_reduce over the expert axis.
  - Attention (head c = core c) runs in transposed-A form: logits^T = K Q^T
    per key-chunk, exp on the scalar engine straight into SBUF (no max
    subtraction -- logits are bounded), A^T feeds V^T A^T directly.  The
    softmax denominator rides along as a 65th ones-column of V, and the
    normalization happens once on attn_out^T via a DMA-broadcast reciprocal.
  - Expand-router scores: partial contraction of the local normalized
    attn_out^T with Wo^T rank rows; per-batch AllReduce sums them.  Per-batch
    AllGathers distribute attn_out^T [512, 2048]; batch-0 collectives hide
    under batch-1's attention, batch-1 collectives under batch-0's expand.
  - d_model-sharded expand: core c owns output columns [128c, 128c+128),
    dense over all 32 experts, top-4 gated combine split across the gpsimd
    (SBUF copies) and vector engines so the expand phase stays tensor-bound.

Precision: top-k selection flips are the dominant error mode and need
~1e-6-level score accuracy, so everything feeding a router runs at
fp32-grade precision: compress scores and the compress main matmul use a
manual fp16 hi/lo split (3 fp16 passes; products are exact in the PE's FP22
pipeline), attention and expand partial scores use true-fp32 matmuls.  The
expand main matmul is post-routing and linear in the output, so it runs in
float32r (measured ~1e-4 max rel err, well inside the 2e-2 gate).
"""

from contextlib import ExitStack

import numpy as np

import concourse.bass as bass  # noqa: F401
import concourse.mybir as mybir
import concourse.tile as tile
from concourse import bacc
from concourse.bass_utils import run_bass_kernel_spmd

F32 = mybir.dt.float32
F32R = mybir.dt.float32r
F16 = mybir.dt.float16
AX = mybir.AxisListType
OP = mybir.AluOpType
AF = mybir.ActivationFunctionType

N_CORES = 8
B, S, D, R, H, DH = 2, 1024, 1024, 512, 8, 64
BS = B * S  # 2048 tokens
NEXP = 32
TCH = BS // 128  # 16 token chunks
KD = D // 128  # 8 k-tiles over d_model
KR = R // 128  # 4 k-tiles over rank
NQ = S // 128  # 8 key chunks per batch
NEG = -1e30
USE_GP = True  # offload half the expand gate-multiplies to gpsimd


def _build_program():
    nc = bacc.Bacc(
        "TRN2", target_bir_lowering=False, debug=False, num_devices=N_CORES
    )
    io = dict(
        xth=nc.dram_tensor("xth", [D, BS], F16, kind="ExternalInput"),
        xtl=nc.dram_tensor("xtl", [D, BS], F16, kind="ExternalInput"),
        cwh=nc.dram_tensor("cwh", [128, KD, NEXP * DH], F16, kind="ExternalInput"),
        cwl=nc.dram_tensor("cwl", [128, KD, NEXP * DH], F16, kind="ExternalInput"),
        ew=nc.dram_tensor("ew", [128, KR, NEXP * 128], F32R, kind="ExternalInput"),
        wrh=nc.dram_tensor("wrh", [128, KD, 96], F16, kind="ExternalInput"),
        wrl=nc.dram_tensor("wrl", [128, KD, 96], F16, kind="ExternalInput"),
        wol=nc.dram_tensor("wol", [64, 32], F32, kind="ExternalInput"),
        ident=nc.dram_tensor("ident", [128, 128], F32, kind="ExternalInput"),
        causalt=nc.dram_tensor("causalt", [128, 128], F32, kind="ExternalInput"),
        outt=nc.dram_tensor("outt", [TCH, 128, 128], F32, kind="ExternalOutput"),
    )
    with tile.TileContext(nc) as tc:
        _emit(nc, tc, io)
    nc.compile()
    return nc


def _emit(nc, tc, io):
    with ExitStack() as ctx:
        glob = ctx.enter_context(tc.tile_pool(name="glob", bufs=1))
        dr = ctx.enter_context(tc.tile_pool(name="dram", bufs=1, space="DRAM"))
        # one uniform rotating PSUM pool: 4 bufs x [128, 1024] f32 = all 8 banks
        psb = ctx.enter_context(tc.tile_pool(name="psb", bufs=4, space="PSUM"))

        IDENT = glob.tile([128, 128], F32, tag="ident")
        nc.sync.dma_start(IDENT[:], io["ident"][:])
        CAUSALT = glob.tile([128, 128], F32, tag="causalt")
        nc.sync.dma_start(CAUSALT[:], io["causalt"][:])
        # ACC_v carries a 65th all-ones column: row 64 of V^T A^T is then the
        # softmax denominator Z for free.
        ACC = {}
        for p in "qk":
            ACC[p] = glob.tile([128, TCH, DH], F32, tag=f"acc_{p}", name=f"acc_{p}")
        ACC["v"] = glob.tile([128, TCH, DH + 1], F32, tag="acc_v", name="acc_v")
        nc.vector.memset(ACC["v"][:, :, DH : DH + 1], 1.0)

        # ============ Phase A: scores + gating + compress + transposes ========
        with (
            tc.tile_pool(name="pa", bufs=1) as pa,
            tc.tile_pool(name="pa_s", bufs=3) as pas,
        ):
            WRH = pa.tile([128, KD, 96], F16, tag="wrh")
            nc.sync.dma_start(WRH[:], io["wrh"][:])
            WRL = pa.tile([128, KD, 96], F16, tag="wrl")
            nc.sync.dma_start(WRL[:], io["wrl"][:])
            XTH = pa.tile([128, KD, BS], F16, tag="xth")
            XTL = pa.tile([128, KD, BS], F16, tag="xtl")
            for kt in range(KD):  # per-ktile DMAs so matmuls start early
                nc.sync.dma_start(
                    XTH[:, kt, :], io["xth"][kt * 128 : (kt + 1) * 128, :]
                )
                nc.sync.dma_start(
                    XTL[:, kt, :], io["xtl"][kt * 128 : (kt + 1) * 128, :]
                )
            CWH = pa.tile([128, KD, NEXP * DH], F16, tag="cwh")
            CWL = pa.tile([128, KD, NEXP * DH], F16, tag="cwl")
            for kt in range(KD):
                nc.sync.dma_start(CWH[:, kt, :], io["cwh"][:, kt, :])
                nc.sync.dma_start(CWL[:, kt, :], io["cwl"][:, kt, :])

            # ---- compress router scores (fp16-split) interleaved with chunk 0
            # scoresT [96, 2048] split over two psum tiles (tokens 0:1024 |
            # 1024:2048) so the whole program shares one uniform psum shape.
            sc = [psb.tile([128, 1024], F32, tag="b", name=f"sc{h}") for h in (0, 1)]
            ch0 = [psb.tile([128, 1024], F32, tag="b", name=f"ch0_{h}") for h in (0, 1)]
            cterms = ((XTH, CWH), (XTH, CWL), (XTL, CWH))
            sterms = ((WRH, XTH), (WRH, XTL), (WRL, XTH))
            for kt in range(KD):
                for ti, (wt, xt_) in enumerate(sterms):
                    for nch in range(4):
                        nc.tensor.matmul(
                            sc[nch // 2][:96, (nch % 2) * 512 : (nch % 2 + 1) * 512],
                            lhsT=wt[:, kt, :],
                            rhs=xt_[:, kt, nch * 512 : (nch + 1) * 512],
                            start=(kt == 0 and ti == 0),
                            stop=(kt == KD - 1 and ti == 2),
                        )
                for ti, (xt_, cw_) in enumerate(cterms):
                    for g in range(4):
                        nc.tensor.matmul(
                            ch0[g // 2][:, (g % 2) * 512 : (g % 2 + 1) * 512],
                            lhsT=xt_[:, kt, 0:128],
                            rhs=cw_[:, kt, g * 512 : (g + 1) * 512],
                            start=(kt == 0 and ti == 0),
                            stop=(kt == KD - 1 and ti == 2),
                        )
            ST = pa.tile([96, BS], F32, tag="scoresT")
            nc.scalar.copy(ST[:, 0:1024], sc[0][:96, :])
            nc.scalar.copy(ST[:, 1024:2048], sc[1][:96, :])
            # transpose to SCORES [128, TCH, 96] (chunk-major: q|k|v per chunk)
            SCORES = pa.tile([128, TCH, 96], F32, tag="scores")
            for half in range(2):
                pt = psb.tile([128, 1024], F32, tag="b", name=f"st_t{half}")
                # 128-aligned slots: a 96-wide transpose output must not
                # cross a 512-column psum bank boundary
                for i in range(8):
                    nc.tensor.transpose(
                        pt[:, i * 128 : i * 128 + 96],
                        ST[:, (half * 8 + i) * 128 : (half * 8 + i + 1) * 128],
                        IDENT[:96, :96],
                    )
                nc.scalar.copy(
                    SCORES[:, half * 8 : half * 8 + 8, :],
                    pt[:].rearrange("p (c w) -> p c w", w=128)[:, :, 0:96],
                )

            # ---- gating: top-8 of 32 for q/k/v, emitted in two halves ----
            NROW = TCH * 3  # row j = chunk*3 + proj
            T8 = pa.tile([128, NROW * 8], F32, tag="t8")
            WORK = pa.tile([128, NROW, 32], F32, tag="workc")
            GATES = pa.tile([128, NROW, 32], F32, tag="gates")
            Z = pa.tile([128, NROW], F32, tag="zc")
            RZ = pa.tile([128, NROW], F32, tag="rzc")
            SCF = SCORES[:].rearrange("p c n -> p (c n)")  # [128, 1536]

            def emit_gating_half(h):
                j0, j1 = h * 24, h * 24 + 24
                for j in range(j0, j1):
                    nc.vector.max(
                        T8[:, j * 8 : j * 8 + 8], SCF[:, j * 32 : (j + 1) * 32]
                    )
                    nc.vector.match_replace(
                        WORK[:, j, :],
                        in_to_replace=T8[:, j * 8 : j * 8 + 8],
                        in_values=SCF[:, j * 32 : (j + 1) * 32],
                        imm_value=NEG,
                    )
                SC3 = SCORES[:].rearrange("p c n -> p (c n)").rearrange(
                    "p (j n) -> p j n", n=32
                )[:, j0:j1]
                WKF = WORK[:, j0:j1].rearrange("p j n -> p (j n)")
                SCH = SCF[:, j0 * 32 : j1 * 32]
                # sel mask in-place into WORK: >=1 at top-8 positions, 0 else
                nc.vector.tensor_sub(WKF, SCH, WKF)
                nc.vector.tensor_scalar_min(WKF, WKF, 1.0)
                # gates = exp(s - m) * sel / Z
                M1 = T8[:].rearrange("p (j e) -> p j e", e=8)[:, j0:j1, 0:1]
                GH = GATES[:, j0:j1]
                GHF = GH.rearrange("p j n -> p (j n)")
                nc.vector.tensor_tensor(
                    GH, SC3, M1.to_broadcast([128, 24, 32]), op=OP.subtract
                )
                nc.scalar.activation(GHF, GHF, AF.Exp)
                nc.vector.tensor_mul(GHF, GHF, WKF)
                nc.vector.tensor_reduce(
                    Z[:, j0:j1], GH, axis=AX.X, op=OP.add
                )
                nc.vector.reciprocal(RZ[:, j0:j1], Z[:, j0:j1])
                nc.vector.tensor_tensor(
                    GH,
                    GH,
                    RZ[:, j0:j1, None].to_broadcast([128, 24, 32]),
                    op=OP.mult,
                )

            emit_gating_half(0)

            # ---- compress main (fp16-split) + gated combine ----
            def emit_combine(i, tiles):
                for pi, p in enumerate("qkv"):
                    stg = pas.tile([128, 2048], F32, tag="stage_c")
                    for h in range(2):
                        gv = GATES[:, i * 3 + pi, h * 16 : (h + 1) * 16, None]
                        nc.vector.tensor_tensor(
                            stg[:, h * 1024 : (h + 1) * 1024].rearrange(
                                "p (n r) -> p n r", r=DH
                            ),
                            tiles[h][:].rearrange("p (n r) -> p n r", r=DH),
                            gv.to_broadcast([128, 16, DH]),
                            op=OP.mult,
                        )
                    nc.vector.tensor_reduce(
                        ACC[p][:, i, 0:DH],
                        stg[:].rearrange("p (n r) -> p r n", r=DH),
                        axis=AX.X,
                        op=OP.add,
                    )

            if "d_scores" in io:
                nc.sync.dma_start(io["d_scores"][:], SCORES[:])

            emit_combine(0, ch0)
            for i in range(1, TCH):
                chp = [
                    psb.tile([128, 1024], F32, tag="b", name=f"ch{i}_{h}")
                    for h in (0, 1)
                ]
                tsl = slice(i * 128, (i + 1) * 128)
                for kt in range(KD):
                    for ti, (xt_, cw_) in enumerate(cterms):
                        for g in range(4):
                            nc.tensor.matmul(
                                chp[g // 2][:, (g % 2) * 512 : (g % 2 + 1) * 512],
                                lhsT=xt_[:, kt, tsl],
                                rhs=cw_[:, kt, g * 512 : (g + 1) * 512],
                                start=(kt == 0 and ti == 0),
                                stop=(kt == KD - 1 and ti == 2),
                            )
                emit_combine(i, chp)
                if i == 1:
                    emit_gating_half(1)
            if "d_gates" in io:
                nc.sync.dma_start(io["d_gates"][:], GATES[:])

        # ============ Phase B/C: attention + expand, per batch ================
        pc = ctx.enter_context(tc.tile_pool(name="pc", bufs=1))
        EW = pc.tile([128, KR, NEXP * 128], F32R, tag="ew")
        nc.sync.dma_start(EW[:], io["ew"][:])  # after pa closes: SBUF is free
        WOL = pc.tile([64, 32], F32, tag="wol")
        nc.sync.dma_start(WOL[:], io["wol"][:])
        ATF = pc.tile([128, KR, BS], F32R, tag="attnT_full")
        QT = pc.tile([64, BS], F32, tag="qt")
        KT = pc.tile([64, BS], F32, tag="kt")
        ATL = pc.tile([64, BS], F32, tag="attnT_local")
        SOT = pc.tile([32, BS], F32, tag="soT")
        SCO = pc.tile([128, TCH, 32], F32, tag="sco")
        GO = pc.tile([128, TCH, 32], F32, tag="go")
        RZS = pc.tile([64, 1024], F32, tag="rzs")
        ZS = pc.tile([65, 1024], F32, tag="zs")
        ZB = pc.tile([128, 8], F32, tag="zb")
        RZB = pc.tile([128, 8], F32, tag="rzb")
        RZROW = pc.tile([1, 1024], F32, tag="rzrow")

        pbs = ctx.enter_context(tc.tile_pool(name="pb_s", bufs=3))
        pcs = ctx.enter_context(tc.tile_pool(name="pc_s", bufs=2))

        def emit_qkt_transposes(half):
            """Transpose ACC q/k chunks [8h, 8h+8) into QT/KT columns."""
            tq = psb.tile([64, 1024], F32, tag="b", name=f"tq{half}")
            tk = psb.tile([64, 1024], F32, tag="b", name=f"tk{half}")
            for j in range(8):
                i = half * 8 + j
                nc.tensor.transpose(
                    tq[:, j * 128 : (j + 1) * 128], ACC["q"][:, i, :], IDENT[:]
                )
                nc.tensor.transpose(
                    tk[:, j * 128 : (j + 1) * 128], ACC["k"][:, i, :], IDENT[:]
                )
            nc.scalar.copy(QT[:, half * 1024 : (half + 1) * 1024], tq[:])
            nc.scalar.copy(KT[:, half * 1024 : (half + 1) * 1024], tk[:])

        def emit_attention(b):
            off = b * S
            psO = psb.tile([128, 1024], F32, tag="b", name=f"psO{b}")
            for ki in range(NQ):
                k0 = ki * 128
                W = S - k0
                psL = psb.tile([128, 1024], F32, tag="b", name=f"psL{b}_{ki}")
                # QK pieces: align to psL's local 512-column psum banks
                for llo in range(0, W, 512):
                    lhi = min(llo + 512, W)
                    nc.tensor.matmul(
                        psL[:, llo:lhi],
                        lhsT=KT[:, off + k0 : off + k0 + 128],
                        rhs=QT[:, off + k0 + llo : off + k0 + lhi],
                        start=True,
                        stop=True,
                    )
                # AV pieces: align to psO's global 512-column psum banks
                pieces = []
                lo = k0
                while lo < S:
                    hi = min((lo // 512 + 1) * 512, S)
                    pieces.append((lo, hi))
                    lo = hi
                # causal mask on the diagonal block (keep q >= k)
                nc.vector.tensor_add(
                    psL[:, 0:128], psL[:, 0:128], CAUSALT[:]
                )
                EXPA = pbs.tile([128, 1024], F32, tag="expa")
                nc.scalar.activation(
                    EXPA[:, 0:W], psL[:, 0:W], AF.Exp, scale=0.125
                )
                for lo, hi in pieces:
                    nc.tensor.matmul(
                        psO[0:65, lo:hi],
                        lhsT=ACC["v"][:, b * NQ + ki, :],
                        rhs=EXPA[:, lo - k0 : hi - k0],
                        start=(ki == 0),
                        stop=(ki == NQ - 1),
                    )
            # normalization: rz = 1/Z broadcast to rank rows via DMA
            nc.scalar.copy(ZS[64:65, :], psO[64:65, :])
            nc.sync.dma_start(ZB[:], ZS[64:65, :])
            nc.vector.reciprocal(RZB[:], ZB[:])
            nc.sync.dma_start(RZROW[:], RZB[:])
            nc.gpsimd.partition_broadcast(RZS[0:64, :], RZROW[0:1, :])
            nc.vector.tensor_tensor(
                ATL[:, off : off + S], psO[0:64, :], RZS[:], op=OP.mult
            )

            # partial expand-router scores for this batch (true fp32)
            ps_q = psb.tile([128, 1024], F32, tag="b", name=f"ps_q{b}")
            for ncb in range(2):
                nc.tensor.matmul(
                    ps_q[0:32, ncb * 512 : (ncb + 1) * 512],
                    lhsT=WOL[:],
                    rhs=ATL[:, off + ncb * 512 : off + (ncb + 1) * 512],
                    start=True,
                    stop=True,
                )
            SQ = pbs.tile([32, 1024], F32, tag="sq", bufs=2)
            nc.scalar.copy(SQ[:], ps_q[0:32, :])

            # per-batch collectives: AllReduce (small, first) then AllGather
            bi_ar = dr.tile([32, S], F32, name=f"bi_ar{b}")
            bo_ar = dr.tile([32, S], F32, addr_space="Shared", name=f"bo_ar{b}")
            nc.sync.dma_start(bi_ar[:], SQ[:])
            nc.gpsimd.collective_compute(
                "AllReduce",
                OP.add,
                replica_groups=[list(range(N_CORES))],
                ins=[bi_ar[:]],
                outs=[bo_ar[:]],
            )
            bi_ag = dr.tile([64, S], F32R, name=f"bi_ag{b}")
            bo_ag = dr.tile(
                [N_CORES * 64, S], F32R, addr_space="Shared", name=f"bo_ag{b}"
            )
            # f32 -> f32r cast on the small (pre-gather) side via SWDGE
            nc.gpsimd.dma_start(bi_ag[:], ATL[:, off : off + S])
            nc.gpsimd.collective_compute(
                "AllGather",
                OP.bypass,
                replica_groups=[list(range(N_CORES))],
                ins=[bi_ag[:]],
                outs=[bo_ag[:]],
            )
            nc.sync.dma_start(SOT[:, off : off + S], bo_ar[:])
            nc.sync.dma_start(
                ATF[:, :, off : off + S],
                bo_ag[:].rearrange("(k p) t -> p k t", p=128),
            )

        def emit_expand_gating(b):
            # SCO transposes + top-4 gating for this batch's 8 chunks
            ptc = psb.tile([128, 1024], F32, tag="b", name=f"sco_t{b}")
            for j in range(8):
                i = b * NQ + j
                nc.tensor.transpose(
                    ptc[:, j * 32 : (j + 1) * 32],
                    SOT[:, i * 128 : (i + 1) * 128],
                    IDENT[:32, :32],
                )
            nc.scalar.copy(SCO[:, b * NQ : b * NQ + 8, :], ptc[:, 0:256])

            j0, j1 = b * NQ, b * NQ + 8
            T8O = pc.tile([128, TCH * 8], F32, tag="t8o", name=f"t8o_{b}")
            WKO = pc.tile([128, TCH, 32], F32, tag="worko", name=f"wko_{b}")
            SCOF = SCO[:].rearrange("p c n -> p (c n)")
            for i in range(j0, j1):
                nc.vector.max(
                    T8O[:, i * 8 : i * 8 + 8], SCOF[:, i * 32 : (i + 1) * 32]
                )
            T8OV = T8O[:].rearrange("p (i e) -> p i e", e=8)
            nc.vector.memset(T8OV[:, j0:j1, 4:8], 1e30)
            for i in range(j0, j1):
                nc.vector.match_replace(
                    WKO[:, i, :],
                    in_to_replace=T8O[:, i * 8 : i * 8 + 8],
                    in_values=SCOF[:, i * 32 : (i + 1) * 32],
                    imm_value=NEG,
                )
            WKH = WKO[:, j0:j1].rearrange("p i n -> p (i n)")
            SCH = SCOF[:, j0 * 32 : j1 * 32]
            nc.vector.tensor_sub(WKH, SCH, WKH)
            nc.vector.tensor_scalar_min(WKH, WKH, 1.0)
            MO = T8OV[:, j0:j1, 0:1]
            GH = GO[:, j0:j1]
            GHF = GH.rearrange("p i n -> p (i n)")
            nc.vector.tensor_tensor(
                GH, SCO[:, j0:j1], MO.to_broadcast([128, 8, 32]), op=OP.subtract
            )
            nc.scalar.activation(GHF, GHF, AF.Exp)
            nc.vector.tensor_mul(GHF, GHF, WKH)
            ZO = pc.tile([128, 8], F32, tag="zo", name=f"zo_{b}")
            nc.vector.tensor_reduce(ZO[:], GH, axis=AX.X, op=OP.add)
            RZO = pc.tile([128, 8], F32, tag="rzo", name=f"rzo_{b}")
            nc.vector.reciprocal(RZO[:], ZO[:])
            nc.vector.tensor_tensor(
                GH, GH, RZO[:, :, None].to_broadcast([128, 8, 32]), op=OP.mult
            )

        def emit_expand(b):
            off = b * S
            for j in range(NQ):
                i = b * NQ + j
                tsl = slice(off + j * 128, off + (j + 1) * 128)
                qt_tiles = []
                SC4 = pcs.tile([128, 4096], F32, tag="sc4", bufs=1)
                CP = [None, None]
                for q in range(4):
                    qt_ = psb.tile([128, 1024], F32, tag="b", name=f"ex{i}_{q}")
                    qt_tiles.append(qt_)
                    for kt in range(KR):
                        for half in range(2):
                            cbase = q * 1024 + half * 512
                            nc.tensor.matmul(
                                qt_[:, half * 512 : (half + 1) * 512],
                                lhsT=ATF[:, kt, tsl],
                                rhs=EW[:, kt, cbase : cbase + 512],
                                start=(kt == 0),
                                stop=(kt == KR - 1),
                            )
                    if USE_GP and q < 2:
                        CP[q] = pcs.tile(
                            [128, 1024], F32, tag="cp", bufs=2, name=f"cp{i}_{q}"
                        )
                        nc.scalar.copy(CP[q][:], qt_[:])
                # gate-multiply: quarters 0,1 on gpsimd (from SBUF copies),
                # quarters 2,3 on the vector engine straight from PSUM
                for q in range(4):
                    gv = GO[:, i, q * 8 : (q + 1) * 8, None]
                    src = CP[q] if (USE_GP and q < 2) else qt_tiles[q]
                    eng = nc.gpsimd if (USE_GP and q < 2) else nc.vector
                    eng.tensor_tensor(
                        SC4[:, q * 1024 : (q + 1) * 1024].rearrange(
                            "p (n r) -> p n r", r=128
                        ),
                        src[:].rearrange("p (n r) -> p n r", r=128),
                        gv.to_broadcast([128, 8, 128]),
                        op=OP.mult,
                    )
                OC = pcs.tile([128, 128], F32, tag="oc")
                nc.vector.tensor_reduce(
                    OC[:],
                    SC4[:].rearrange("p (n r) -> p r n", r=128),
                    axis=AX.X,
                    op=OP.add,
                )
                nc.sync.dma_start(io["outt"][i, :, :], OC[:])

        emit_qkt_transposes(0)
        emit_attention(0)
        emit_qkt_transposes(1)
        emit_attention(1)
        emit_expand_gating(0)
        emit_expand(0)
        emit_expand_gating(1)
        emit_expand(1)

        if "d_accq" in io:
            nc.sync.dma_start(io["d_accq"][:], ACC["q"][:])
            nc.sync.dma_start(io["d_accv"][:], ACC["v"][:])
            nc.sync.dma_start(io["d_atl"][:], ATL[:])
            nc.sync.dma_start(io["d_sot"][:], SOT[:])
            nc.sync.dma_start(io["d_go"][:], GO[:])


_PROGRAM = None


def _get_program():
    global _PROGRAM
    if _PROGRAM is None:
        _PROGRAM = _build_program()
    return _PROGRAM


def _hilo(a32):
    """fp16 hi/lo split: a32 ~= hi + lo with the product path exact in FP22."""
    hi = a32.astype(np.float16)
    lo = (a32 - hi.astype(np.float32)).astype(np.float16)
    return np.ascontiguousarray(hi), np.ascontiguousarray(lo)


def _prep_inputs(x, compress_neurons, expand_neurons, Wq, Wk, Wv, Wo):
    """Build the 8 per-core input maps (numpy, DMA-friendly layouts)."""
    X = np.ascontiguousarray(x.reshape(BS, D), dtype=np.float32)
    xt = np.ascontiguousarray(X.T)  # [D, BS]
    xth, xtl = _hilo(xt)
    wr = (
        np.stack([Wq, Wk, Wv], axis=0)  # [3, 32, D]
        .transpose(2, 0, 1)  # [D, 3, 32]
        .reshape(D, 96)
        .reshape(KD, 128, 96)
        .transpose(1, 0, 2)  # [128, KD, 96]
    )
    wr = np.ascontiguousarray(wr, dtype=np.float32)
    wrh, wrl = _hilo(wr)
    ident = np.eye(128, dtype=np.float32)
    # causalt[k, q] = 0 if q >= k else NEG  (transposed-A layout)
    causalt = np.where(
        np.arange(128)[None, :] >= np.arange(128)[:, None], 0.0, NEG
    ).astype(np.float32)

    in_maps = []
    for c in range(N_CORES):
        cwc = compress_neurons[:, :, c * DH : (c + 1) * DH]  # [32, D, 64]
        cw = np.ascontiguousarray(
            cwc.reshape(NEXP, KD, 128, DH)
            .transpose(2, 1, 0, 3)  # [128, KD, 32, 64]
            .reshape(128, KD, NEXP * DH),
            dtype=np.float32,
        )
        cwh, cwl = _hilo(cw)
        ewc = expand_neurons[:, :, c * 128 : (c + 1) * 128]  # [32, R, 128]
        ew = np.ascontiguousarray(
            ewc.reshape(NEXP, KR, 128, 128)
            .transpose(2, 1, 0, 3)  # [128, KR, 32, 128]
            .reshape(128, KR, NEXP * 128),
            dtype=np.float32,
        )
        wol = np.ascontiguousarray(Wo[:, c * DH : (c + 1) * DH].T, dtype=np.float32)
        in_maps.append(
            dict(
                xth=xth,
                xtl=xtl,
                cwh=cwh,
                cwl=cwl,
                ew=ew,
                wrh=wrh,
                wrl=wrl,
                wol=wol,
                ident=ident,
                causalt=causalt,
            )
        )
    return in_maps


def kernel(x, mask, compress_neurons, expand_neurons, Wq, Wk, Wv, Wo):
    """Full-input entry point; returns the [B, S, D] fp32 output."""
    x = np.asarray(x, dtype=np.float32)
    compress_neurons = np.asarray(compress_neurons, dtype=np.float32)
    expand_neurons = np.asarray(expand_neurons, dtype=np.float32)
    Wq, Wk, Wv, Wo = (np.asarray(w, dtype=np.float32) for w in (Wq, Wk, Wv, Wo))

    nc = _get_program()
    in_maps = _prep_inputs(x, compress_neurons, expand_neurons, Wq, Wk, Wv, Wo)
    res = run_bass_kernel_spmd(nc, in_maps, core_ids=list(range(N_CORES)))
    out = np.empty((BS, D), dtype=np.float32)
    for c in range(N_CORES):
        oc = res.results[c]["outt"]  # [TCH, 128, 128]
        out[:, c * 128 : (c + 1) * 128] = oc.reshape(BS, 128)
    return out.reshape(B, S, D)
